# revision 21
# baseline (speedup 1.0000x reference)
"""Trainium2 Bass kernel for nn_CG_MSA_M (cross-gated multi-head channel attention).

Sharding: data-parallel over batch (8 samples -> 8 cores), weights replicated.

Per-core algorithm (one sample, C=96 channels, N=128x128 positions):
  - x,y are staged into zero-padded 130x130 "canvases" (fp16) so every 3x3
    shift is a pure access-pattern offset with exact zero padding.
  - The depthwise 3x3 convs are folded into the producing 1x1 convs on the
    TensorEngine.  To use all 128 PE rows (instead of 96), each 9-tap x
    96-channel group (864 row-units) is covered by SEVEN K<=128 matmuls
    instead of nine:
      P-tile (the canvas, 128 partitions: 96 base + ch0-31 shifted (0,+1))
        streamed at (sy,0), sy=0..2          -> covers (sy,0)x96 + (sy,1)x32
      Q-ring (128 partitions: ch32-95 @ (0,+1), ch0-63 @ (0,+2))
        streamed at (sy,0), sy=0..2          -> covers (sy,1)x64 + (sy,2)x64
      R-ring (96 partitions: ch64-95 @ {(0,2),(1,2),(2,2)})
        streamed once                        -> covers (sy,2)x32 for all sy
    Q/R are 16/18-row rolling rings filled by SBUF->SBUF DMA from the canvas
    (the DMA engines are otherwise underutilized).
  - Same 7-MM cover for the fuse 3x3 conv (two 96-ch halves: v0, v_).
  - Channel-attention Gram matrices (contract over the 16384 positions) are
    accumulated chunk-wise via DMA transposes + matmuls; L2 norms come from
    Square-activation accumulation.
  - v is spilled to DRAM (bf16) and re-streamed in the output phase.
  - Output = [W_proj@A | W_posX | W_posY] @ [v; x; y] fused into one matmul
    group per chunk, DMA'd straight from PSUM-staging to DRAM.
"""

import os
import numpy as np
import ml_dtypes

import concourse.bass as bass
import concourse.tile as tile
from concourse import bacc, mybir
from concourse.bass_utils import run_bass_kernel_spmd

BF16 = mybir.dt.float16  # fp16: same PE rate as bf16, 10-bit mantissa
F32 = mybir.dt.float32

B, C, H, W, HEADS = 8, 96, 128, 128, 6
CH = C // HEADS  # 16
N = H * W  # 16384
WC = W + 2  # canvas row stride 130
CANV = (H + 2) * WC  # 16900
NCHUNK = 512  # positions per chunk (4 rows)
RPC = NCHUNK // W  # rows per chunk = 4
NCB = N // NCHUNK  # 32 chunks

RRING = 16         # R-ring rows (phys = canvas_row mod 16)
QRING = RRING + 2  # Q-ring rows: +2 dup rows mirroring phys 0,1

# module-level knob so test.py can request a profiled run
TRACE = False
LAST_RESULTS = None


def _bf16(a):
    return np.ascontiguousarray(a.astype(np.float16))


def _f32(a):
    return np.ascontiguousarray(a.astype(np.float32))


def _group_weights(A):
    """A: [9, 96 in, 96 out] per-tap lhsT blocks (t = dy*3+dx).
    Returns the P/Q/R lhsT tensors for the 7-MM cover."""
    wP = np.zeros((128, 3, C), np.float32)
    wQ = np.zeros((128, 3, C), np.float32)
    wR = np.zeros((C, C), np.float32)
    for sy in range(3):
        wP[0:96, sy] = A[sy * 3 + 0]
        wP[96:128, sy] = A[sy * 3 + 1][0:32]
        wQ[0:64, sy] = A[sy * 3 + 1][32:96]
        wQ[64:128, sy] = A[sy * 3 + 2][0:64]
    for b in range(3):
        wR[32 * b:32 * b + 32] = A[b * 3 + 2][64:96]
    return (_bf16(wP.reshape(128, 3 * C)), _bf16(wQ.reshape(128, 3 * C)),
            _bf16(wR))


def _prep_weights(w_pos, w_qv, w_qv_dw, w_kv, w_kv_dw, w_proj, w_fuse, b_fuse,
                  temperature):
    """Host-side weight composition (numpy)."""
    w_pos = w_pos[:, :, 0, 0]      # [192,192]
    w_qv = w_qv[:, :, 0, 0]        # [192,96]
    w_kv = w_kv[:, :, 0, 0]        # [192,96]
    w_proj = w_proj[:, :, 0, 0]    # [192,96]
    dwq = w_qv_dw[:, 0].reshape(2 * C, 9)   # [192,9]
    dwk = w_kv_dw[:, 0].reshape(2 * C, 9)   # [192,9]

    out = {}
    # conv groups: A_t[i,o] = W[o,i] * dw[o,t]
    for nm, Wg, dwg in (("q", w_qv[:C], dwq[:C]), ("vx", w_qv[C:], dwq[C:]),
                        ("k", w_kv[:C], dwk[:C]), ("vy", w_kv[C:], dwk[C:])):
        A = np.einsum('oi,ot->tio', Wg, dwg)
        p, q, r = _group_weights(A)
        out[f"wP_{nm}"], out[f"wQ_{nm}"], out[f"wR_{nm}"] = p, q, r
    # fuse groups: A_t[j,o] = w_fuse[o, 96*half + j, t]
    wfr = w_fuse.reshape(C, 2 * C, 9)
    for nm, half in (("f0", 0), ("f_", 1)):
        A = np.transpose(wfr[:, half * C:(half + 1) * C, :], (2, 1, 0))
        p, q, r = _group_weights(np.ascontiguousarray(A))
        out[f"wP_{nm}"], out[f"wQ_{nm}"], out[f"wR_{nm}"] = p, q, r

    out["wprojT"] = _bf16(w_proj.T)          # [96 c', 192 o]
    out["wposxT"] = _bf16(w_pos[:, :C].T)    # [96 i, 192 o]
    out["wposyT"] = _bf16(w_pos[:, C:].T)    # [96 i, 192 o]
    out["temp_row"] = _f32(np.repeat(temperature.reshape(HEADS), CH)
                           .reshape(C, 1))
    out["bfuse"] = _f32(b_fuse.reshape(C, 1))
    out["identf"] = _f32(np.eye(C))
    out["mask"] = _f32(np.kron(np.eye(HEADS), np.ones((CH, CH))) * 30000.0
                       - 30000.0)
    return out


def _dummy_out(tc, nc, out_d):
    with tc.tile_pool(name="dummy", bufs=2) as dp:
        for mt, (o0, osz) in enumerate(((0, 128), (128, 64))):
            for cb in range(NCB):
                t = dp.tile([osz, NCHUNK], F32, tag=f"d{mt}")
                nc.vector.memset(t[:], 0.0)
                nc.sync.dma_start(
                    out=out_d[o0:o0 + osz, cb * NCHUNK:(cb + 1) * NCHUNK],
                    in_=t[:])


def _build_nc(repeat=1):
    nc = bacc.Bacc(None, name="cg_msa")

    x_d = nc.dram_tensor("x", [C, N], F32, kind="ExternalInput")
    y_d = nc.dram_tensor("y", [C, N], F32, kind="ExternalInput")
    w_d = {}
    wspec = []
    for g in ("q", "vx", "k", "vy", "f0", "f_"):
        wspec += [(f"wP_{g}", [128, 3 * C], BF16),
                  (f"wQ_{g}", [128, 3 * C], BF16),
                  (f"wR_{g}", [C, C], BF16)]
    wspec += [("wprojT", [C, 2 * C], BF16), ("wposxT", [C, 2 * C], BF16),
              ("wposyT", [C, 2 * C], BF16), ("temp_row", [C, 1], F32),
              ("bfuse", [C, 1], F32), ("identf", [C, C], F32),
              ("mask", [C, C], F32)]
    for nm, shp, dt in wspec:
        w_d[nm] = nc.dram_tensor(nm, shp, dt, kind="ExternalInput")
    out_d = nc.dram_tensor("out", [2 * C, N], F32, kind="ExternalOutput")
    v_d = nc.dram_tensor("vtmp", [C, N], BF16, kind="Internal")

    with tile.TileContext(nc) as tc:
        for rep in range(repeat):
            _emit(tc, nc, x_d, y_d, w_d, out_d, v_d, warmup=(rep == 0))
    nc.finalize()
    return nc


def _ring_runs(r0, r1):
    """Split canvas rows [r0, r1] into contiguous phys runs of the mod-16
    ring.  Returns (phys_start, src_row_start, n) tuples."""
    runs = []
    r = r0
    while r <= r1:
        p = r % RRING
        n = min(RRING - p, r1 - r + 1)
        runs.append((p, r, n))
        r += n
    return runs


def _emit(tc, nc, x_d, y_d, w_d, out_d, v_d, warmup=True):
    from contextlib import ExitStack
    ctx = ExitStack()
    dma_rr = [nc.sync, nc.scalar, nc.gpsimd]
    rrctr = [0]

    def deng():
        e = dma_rr[rrctr[0] % 3]
        rrctr[0] += 1
        return e

    with ctx:
        const = ctx.enter_context(tc.tile_pool(name="const", bufs=1))
        canv = ctx.enter_context(tc.tile_pool(name="canv", bufs=1))
        stats = ctx.enter_context(tc.tile_pool(name="stats", bufs=1))
        cstk = ExitStack()  # phase-C-scoped SBUF pools (rings, io, vstg)
        ringp = cstk.enter_context(tc.tile_pool(name="ringp", bufs=1))

        # ---- weights to SBUF ----
        wsb = {}
        for wi, (nm, t_d) in enumerate(w_d.items()):
            t = const.tile(list(t_d.shape), t_d.dtype, tag=f"w_{nm}")
            (nc.sync if wi % 2 == 0 else nc.scalar).dma_start(
                out=t[:], in_=t_d[:])
            wsb[nm] = t

        # ---- canvases (128 partitions: 96 base + 32 aug) + rings ----
        cvs = {}
        for s in ("x", "y", "v0", "v_"):
            cv = canv.tile([128, CANV], BF16, tag=f"cv_{s}")
            # zero borders on all 128 partitions (covers aug block too)
            nc.gpsimd.memset(cv[:, 0:WC], 0.0)
            nc.gpsimd.memset(cv[:, (H + 1) * WC:CANV], 0.0)
            side = cv[:].rearrange("p (r c) -> p r c", c=WC)
            nc.gpsimd.memset(side[:, 0:H + 1, W + 1:W + 2], 0.0)
            nc.gpsimd.memset(side[:, 1:H + 2, 0:1], 0.0)
            qr = ringp.tile([128, QRING * W], BF16, tag=f"qr_{s}")
            rr = ringp.tile([C, RRING * W], BF16, tag=f"rr_{s}")
            nc.gpsimd.memset(qr[:], 0.0)
            nc.gpsimd.memset(rr[:], 0.0)
            cvs[s] = (cv, qr, rr)

        nqp = stats.tile([C, NCB], F32, tag="nqp")
        nkp = stats.tile([C, NCB], F32, tag="nkp")

        # ---- ring write helpers (SBUF->SBUF DMA from canvas) ----
        def ring_write(s):
            """Propagate canvas rows it*4+1..+4 (fresh chunk rows) into the
            aug block + Q/R rings of source s."""
            def go(it):
                if it < 0 or it >= NCB:
                    return
                cv, qr, rr = cvs[s]
                cvr = cv[:].rearrange("p (r c) -> p r c", c=WC)
                qrv = qr[:].rearrange("p (r c) -> p r c", c=W)
                rrv = rr[:].rearrange("p (r c) -> p r c", c=W)
                r0 = it * RPC + 1
                # P-aug: ch0-31 shifted (0,+1) into partitions 96-127
                deng().dma_start(out=cvr[96:128, r0:r0 + RPC, 0:W],
                                 in_=cvr[0:32, r0:r0 + RPC, 1:1 + W])
                # Q-ring: ring row r <- canvas row r
                for pb, (sp, c0) in enumerate(((32, 1), (0, 2))):
                    for (ph, sr, n) in _ring_runs(r0, r0 + RPC - 1):
                        deng().dma_start(
                            out=qrv[64 * pb:64 * pb + 64, ph:ph + n, :],
                            in_=cvr[sp:sp + 64, sr:sr + n, c0:c0 + W])
                        # dup rows: phys 0,1 mirrored at 16,17
                        for d in range(2):
                            if ph <= d < ph + n:
                                deng().dma_start(
                                    out=qrv[64 * pb:64 * pb + 64,
                                            RRING + d:RRING + d + 1, :],
                                    in_=cvr[sp:sp + 64, sr + d - ph:
                                            sr + d - ph + 1, c0:c0 + W])
                # R-ring: block b ring row r <- canvas row r+b, cols 2..129
                for b in range(3):
                    lo = max(r0 - b, 0)
                    hi = r0 + RPC - 1 - b
                    if hi < lo:
                        continue
                    for (ph, rr_row, n) in _ring_runs(lo, hi):
                        deng().dma_start(
                            out=rrv[32 * b:32 * b + 32, ph:ph + n, :],
                            in_=cvr[64:96, rr_row + b:rr_row + b + n, 2:2 + W])
            return go

        ring_write_x = ring_write("x")
        ring_write_y = ring_write("y")
        ring_write_v0 = ring_write("v0")
        ring_write_v_ = ring_write("v_")

        def end_fixups(srcs):
            """Canvas row 129 (bottom pad, never loaded) maps to Q phys 1
            (dup 17) and R block2 phys 15 — re-zero the stale dups before
            chunk 31 reads them.  Must run after the last ring write that
            touches those rows (canvas row 113, chunk 28) for each source."""
            for s in srcs:
                cv, qr, rr = cvs[s]
                nc.gpsimd.memset(qr[:, (RRING + 1) * W:(RRING + 2) * W], 0.0)
                nc.gpsimd.memset(rr[64:96, (RRING - 1) * W:RRING * W], 0.0)

        # ---- canvas / ring views for matmuls ----
        def pview(cv, cb, sy):
            r = cv[:].rearrange("p (r c) -> p r c", c=WC)
            return r[:, cb * RPC + sy:cb * RPC + sy + RPC, 0:W]

        def bview(cv, cb, dy, dx, p=C):
            r = cv[:].rearrange("p (r c) -> p r c", c=WC)
            return r[0:p, cb * RPC + dy:cb * RPC + dy + RPC, dx:dx + W]

        def qview(qr, cb, sy):
            ph = (cb * RPC + sy) % RRING
            r = qr[:].rearrange("p (r c) -> p r c", c=W)
            return r[:, ph:ph + RPC, :]

        def rview(rr, cb):
            ph = (cb * RPC) % RRING
            r = rr[:].rearrange("p (r c) -> p r c", c=W)
            return r[0:C, ph:ph + RPC, :]

        def tap_mms(ps, g, s, cb, start=True, stop=True):
            """Accumulate the 9-tap folded conv for group g (weights) reading
            source s (canvas+rings) into psum ps: the 7-MM P/Q/R cover."""
            cv, qr, rr = cvs[s]
            for sy in range(3):
                nc.tensor.matmul(ps, wsb[f"wP_{g}"][:, sy * C:(sy + 1) * C],
                                 pview(cv, cb, sy),
                                 start=(start and sy == 0), stop=False)
            for sy in range(3):
                nc.tensor.matmul(ps, wsb[f"wQ_{g}"][:, sy * C:(sy + 1) * C],
                                 qview(qr, cb, sy),
                                 start=False, stop=False)
            nc.tensor.matmul(ps, wsb[f"wR_{g}"][:],
                             rview(rr, cb), start=False, stop=stop)

        # ---- phase B: stream x,y in, convert to fp16 canvases ----
        io = cstk.enter_context(tc.tile_pool(name="io", bufs=6))

        def load_chunk(cb):
            if cb >= NCB:
                return
            for si, (src_d, s) in enumerate(((x_d, "x"), (y_d, "y"))):
                tin = io.tile([C, NCHUNK], F32, tag="in")
                eng = nc.sync if si == 0 else nc.scalar
                eng.dma_start(
                    out=tin[:], in_=src_d[:, cb * NCHUNK:(cb + 1) * NCHUNK])
                dst = bview(cvs[s][0], cb, 1, 1)
                nc.vector.tensor_copy(
                    dst, tin[:].rearrange("p (r c) -> p r c", c=W))

        for cb in range(4):
            load_chunk(cb)

        # PE warmup during the startup bubble
        if warmup:
            with tc.tile_pool(name="warm", bufs=1) as warm, \
                 tc.tile_pool(name="warmps", bufs=1, space="PSUM") as warmps:
                wsc = warm.tile([C, NCHUNK], BF16)
                nc.vector.memset(wsc[:], 0.0)
                wps = warmps.tile([C, NCHUNK], F32)
                for _ in range(16):
                    nc.tensor.matmul(wps[:], wsc[:, 0:C], wsc[:],
                                     start=True, stop=True)

        # ---- phase C ----
        gctx = ExitStack()
        gpool = gctx.enter_context(tc.tile_pool(name="gps", bufs=1,
                                                space="PSUM"))
        g_ps = gpool.tile([C, C], F32)  # Gqk

        vstg_pool = cstk.enter_context(tc.tile_pool(name="vstg", bufs=3))

        def fuse_chunk(psE, cb):
            if cb < 0 or cb >= NCB:
                return
            ps = psE.tile([C, NCHUNK], F32, tag="v")
            tap_mms(ps[:], "f0", "v0", cb, start=True, stop=False)
            tap_mms(ps[:], "f_", "v_", cb, start=False, stop=True)
            vs = vstg_pool.tile([C, NCHUNK], BF16, tag="vs")
            nc.scalar.activation(
                vs[:], ps[:], mybir.ActivationFunctionType.Identity,
                bias=wsb["bfuse"][:], scale=1.0)
            nc.gpsimd.dma_start(
                out=v_d[:, cb * NCHUNK:(cb + 1) * NCHUNK], in_=vs[:])

        with tc.tile_pool(name="psC", bufs=5, space="PSUM") as psC, \
             tc.tile_pool(name="psE", bufs=2, space="PSUM") as psE, \
             tc.tile_pool(name="stC", bufs=6) as stC, \
             tc.tile_pool(name="sqp", bufs=2) as sqp, \
             tc.tile_pool(name="stT", bufs=3) as stT:

            def gram_chunk(tps, cb):
                if tps is None:
                    return
                for j in range(RPC):
                    st = (cb == 0 and j == 0)
                    sp = (cb == NCB - 1 and j == RPC - 1)
                    nc.tensor.matmul(
                        g_ps[:], tps[:, j, 0, :], tps[:, j, 1, :],
                        start=st, stop=sp, skip_group_check=True)

            def qkv_chunk(cb):
                """tap-MM + evac for chunk cb; returns tps tile."""
                if cb < 0 or cb >= NCB:
                    return None
                outs_sb = {}
                for side, (s, gqk, gv) in enumerate(
                        (("x", "q", "vx"), ("y", "k", "vy"))):
                    ps = psC.tile([C, NCHUNK], F32, tag="qv")
                    tap_mms(ps[:], gqk, s, cb)
                    sb = stC.tile([C, NCHUNK], BF16, tag="qk")
                    nc.scalar.copy(out=sb[:], in_=ps[:])
                    sq = sqp.tile([C, NCHUNK], BF16, tag="sq")
                    npart = nqp if side == 0 else nkp
                    nc.scalar.activation(
                        sq[:], sb[:], mybir.ActivationFunctionType.Square,
                        accum_out=npart[:, cb:cb + 1])
                    outs_sb[side] = sb

                    psv = psC.tile([C, NCHUNK], F32, tag="qv")
                    tap_mms(psv[:], gv, s, cb)
                    cvv = cvs["v_"][0] if side == 0 else cvs["v0"][0]
                    nc.vector.tensor_copy(
                        bview(cvv, cb, 1, 1),
                        psv[:].rearrange("p (r c) -> p r c", c=W))

                tps = stT.tile([W, RPC, 2, C], BF16)
                for sd in range(2):
                    nc.scalar.dma_start_transpose(tps[:, :, sd, :],
                                                  outs_sb[sd][:])
                return tps

            tps_hist = {}
            for it in range(NCB + 4):
                load_chunk(it + 4)
                ring_write_x(it)
                ring_write_y(it)
                if it == NCB - 2:
                    end_fixups(("x", "y"))
                tps_hist[it - 1] = qkv_chunk(it - 1)
                ring_write_v0(it - 2)
                ring_write_v_(it - 2)
                if it == NCB:
                    end_fixups(("v0", "v_"))
                fuse_chunk(psE, it - 3)
                gram_chunk(tps_hist.get(it - 2), it - 2)
                tps_hist.pop(it - 3, None)
        cstk.close()

        # ---- phase D: norms, softmax, M1T ----
        smx = ctx.enter_context(tc.tile_pool(name="smx", bufs=1))
        with tc.tile_pool(name="psD", bufs=1, space="PSUM") as psD:
            g_sb = smx.tile([C, C], F32)
            nc.vector.tensor_copy(g_sb[:], g_ps[:])

            rr_ = {}
            for npart, nm in ((nqp, "q"), (nkp, "k")):
                nrm2 = smx.tile([C, 1], F32, tag=f"n{nm}")
                nc.vector.tensor_reduce(
                    nrm2[:], npart[:], axis=mybir.AxisListType.X,
                    op=mybir.AluOpType.add)
                nrm = smx.tile([C, 1], F32, tag=f"s{nm}")
                nc.scalar.sqrt(nrm[:], nrm2[:])
                nc.vector.tensor_scalar_max(nrm[:], nrm[:], 1e-12)
                rinv = smx.tile([C, 1], F32, tag=f"r{nm}")
                nc.vector.reciprocal(rinv[:], nrm[:])
                rr_[nm] = rinv
            nc.vector.tensor_tensor(
                rr_["q"][:], rr_["q"][:], wsb["temp_row"][:],
                mybir.AluOpType.mult)

            rows = {}
            for nm in ("q", "k"):
                rp = psD.tile([1, C], F32, tag="row")
                nc.tensor.transpose(rp[:], rr_[nm][:], wsb["identf"][:])
                rs = smx.tile([1, C], F32, tag=f"row{nm}")
                nc.vector.tensor_copy(rs[:], rp[:])
                rows[nm] = rs
            r_ps = psD.tile([C, C], F32, tag="R")
            nc.tensor.matmul(r_ps[:], rows["q"][:], rows["k"][:])
            logits = smx.tile([C, C], F32)
            nc.vector.tensor_tensor(
                logits[:], g_sb[:], r_ps[:], mybir.AluOpType.mult)
            nc.vector.tensor_tensor(
                logits[:], logits[:], wsb["mask"][:], mybir.AluOpType.add)

            mx = smx.tile([C, 1], F32)
            nc.vector.tensor_reduce(
                mx[:], logits[:], axis=mybir.AxisListType.X,
                op=mybir.AluOpType.max, negate=True)
            e = smx.tile([C, C], F32)
            nc.scalar.activation(
                e[:], logits[:], mybir.ActivationFunctionType.Exp,
                bias=mx[:], scale=1.0)
            s_ = smx.tile([C, 1], F32)
            nc.vector.tensor_reduce(
                s_[:], e[:], axis=mybir.AxisListType.X, op=mybir.AluOpType.add)
            rs = smx.tile([C, 1], F32)
            nc.vector.reciprocal(rs[:], s_[:])
            a_sb = smx.tile([C, C], BF16)
            nc.scalar.mul(a_sb[:], e[:], rs[:])

            m1_ps = psD.tile([C, 2 * C], F32, tag="m1")
            nc.tensor.matmul(m1_ps[:], a_sb[:], wsb["wprojT"][:])
            m1T = smx.tile([C, 2 * C], BF16)
            nc.vector.tensor_copy(m1T[:], m1_ps[:])
        gctx.close()

        # ---- phase F: out = M1 @ v + W_pos @ [x;y] ----
        with tc.tile_pool(name="psF", bufs=4, space="PSUM") as psF, \
             tc.tile_pool(name="vin", bufs=4) as vin, \
             tc.tile_pool(name="ostg", bufs=6) as ostg:
            vin_t = {}

            def vload(cb):
                if cb >= NCB:
                    return
                t = vin.tile([C, NCHUNK], BF16, tag="vin")
                nc.gpsimd.dma_start(
                    out=t[:], in_=v_d[:, cb * NCHUNK:(cb + 1) * NCHUNK])
                vin_t[cb] = t

            for cb in range(2):
                vload(cb)
            for cb in range(NCB):
                vload(cb + 2)
                vt = vin_t.pop(cb)
                for mt, (o0, osz) in enumerate(((0, 128), (128, 64))):
                    ps = psF.tile([osz, NCHUNK], F32, tag=f"o{mt}")
                    nc.tensor.matmul(
                        ps[:], wsb["wposxT"][:, o0:o0 + osz],
                        bview(cvs["x"][0], cb, 1, 1),
                        start=True, stop=False)
                    nc.tensor.matmul(
                        ps[:], wsb["wposyT"][:, o0:o0 + osz],
                        bview(cvs["y"][0], cb, 1, 1),
                        start=False, stop=False)
                    nc.tensor.matmul(
                        ps[:], m1T[:, o0:o0 + osz], vt[:],
                        start=False, stop=True)
                    osb = ostg.tile([osz, NCHUNK], F32, tag=f"os{mt}")
                    if mt == 0:
                        nc.scalar.copy(out=osb[:], in_=ps[:])
                    else:
                        nc.vector.tensor_copy(osb[:], ps[:])
                    oeng = nc.sync if cb % 2 == 0 else nc.scalar
                    oeng.dma_start(
                        out=out_d[o0:o0 + osz,
                                  cb * NCHUNK:(cb + 1) * NCHUNK],
                        in_=osb[:])


_NC_CACHE = None


def kernel(x, y, w_pos, w_qv, w_qv_dw, w_kv, w_kv_dw, w_proj, w_fuse, b_fuse,
           temperature):
    global _NC_CACHE, LAST_RESULTS
    x = _f32(np.asarray(x))
    y = _f32(np.asarray(y))
    wts = _prep_weights(
        np.asarray(w_pos, np.float32), np.asarray(w_qv, np.float32),
        np.asarray(w_qv_dw, np.float32), np.asarray(w_kv, np.float32),
        np.asarray(w_kv_dw, np.float32), np.asarray(w_proj, np.float32),
        np.asarray(w_fuse, np.float32), np.asarray(b_fuse, np.float32),
        np.asarray(temperature, np.float32))

    if _NC_CACHE is None:
        _NC_CACHE = _build_nc()
    nc = _NC_CACHE

    in_maps = []
    for core in range(B):
        m = {"x": np.ascontiguousarray(x[core].reshape(C, N)),
             "y": np.ascontiguousarray(y[core].reshape(C, N))}
        m.update(wts)
        in_maps.append(m)

    res = run_bass_kernel_spmd(nc, in_maps, core_ids=list(range(B)),
                               trace=TRACE)
    LAST_RESULTS = res
    out = np.stack([np.asarray(r["out"]) for r in res.results])
    return out.reshape(B, 2 * C, H, W).astype(np.float32)


if __name__ == "__main__":
    print("built nc ok" if _build_nc() else "")


# revision 22
# speedup vs baseline: 1.1441x; 1.1441x over previous
"""Trainium2 Bass kernel for nn_CG_MSA_M (cross-gated multi-head channel attention).

Sharding: data-parallel over batch (8 samples -> 8 cores), weights replicated.

Per-core algorithm (one sample, C=96 channels, N=128x128 positions):
  - x,y are staged into zero-padded 130x130 "canvases" (fp16) so every 3x3
    shift is a pure access-pattern offset with exact zero padding.
  - The depthwise 3x3 convs are folded into the producing 1x1 convs on the
    TensorEngine.  To use all 128 PE rows (instead of 96), each 9-tap x
    96-channel group (864 row-units) is covered by SEVEN K<=128 matmuls
    instead of nine:
      P-tile (the canvas, 128 partitions: 96 base + ch0-31 shifted (0,+1))
        streamed at (sy,0), sy=0..2          -> covers (sy,0)x96 + (sy,1)x32
      Q-ring (128 partitions: ch32-95 @ (0,+1), ch0-63 @ (0,+2))
        streamed at (sy,0), sy=0..2          -> covers (sy,1)x64 + (sy,2)x64
      R-ring (96 partitions: ch64-95 @ {(0,2),(1,2),(2,2)})
        streamed once                        -> covers (sy,2)x32 for all sy
    Q/R are 16/18-row rolling rings filled by SBUF->SBUF DMA from the canvas
    (the DMA engines are otherwise underutilized).
  - Same 7-MM cover for the fuse 3x3 conv (two 96-ch halves: v0, v_).
  - Channel-attention Gram matrices (contract over the 16384 positions) are
    accumulated chunk-wise via DMA transposes + matmuls; L2 norms come from
    Square-activation accumulation.
  - v is spilled to DRAM (bf16) and re-streamed in the output phase.
  - Output = [W_proj@A | W_posX | W_posY] @ [v; x; y] fused into one matmul
    group per chunk, DMA'd straight from PSUM-staging to DRAM.
"""

import os
import numpy as np
import ml_dtypes

import concourse.bass as bass
import concourse.tile as tile
from concourse import bacc, mybir
from concourse.bass_utils import run_bass_kernel_spmd

BF16 = mybir.dt.float16  # fp16: same PE rate as bf16, 10-bit mantissa
F32 = mybir.dt.float32

B, C, H, W, HEADS = 8, 96, 128, 128, 6
CH = C // HEADS  # 16
N = H * W  # 16384
WC = W + 2  # canvas row stride 130
CANV = (H + 2) * WC  # 16900
NCHUNK = 512  # positions per chunk (4 rows)
RPC = NCHUNK // W  # rows per chunk = 4
NCB = N // NCHUNK  # 32 chunks

RRING = 16         # R-ring rows (phys = canvas_row mod 16)
QRING = RRING + 2  # Q-ring rows: +2 dup rows mirroring phys 0,1

# module-level knob so test.py can request a profiled run
TRACE = False
LAST_RESULTS = None


def _bf16(a):
    return np.ascontiguousarray(a.astype(np.float16))


def _f32(a):
    return np.ascontiguousarray(a.astype(np.float32))


def _group_weights(A):
    """A: [9, 96 in, 96 out] per-tap lhsT blocks (t = dy*3+dx).
    Returns the P/Q/R lhsT tensors for the 7-MM cover."""
    wP = np.zeros((128, 3, C), np.float32)
    wQ = np.zeros((128, 3, C), np.float32)
    wR = np.zeros((C, C), np.float32)
    for sy in range(3):
        wP[0:96, sy] = A[sy * 3 + 0]
        wP[96:128, sy] = A[sy * 3 + 1][0:32]
        wQ[0:64, sy] = A[sy * 3 + 1][32:96]
        wQ[64:128, sy] = A[sy * 3 + 2][0:64]
    for b in range(3):
        wR[32 * b:32 * b + 32] = A[b * 3 + 2][64:96]
    return (_bf16(wP.reshape(128, 3 * C)), _bf16(wQ.reshape(128, 3 * C)),
            _bf16(wR))


def _prep_weights(w_pos, w_qv, w_qv_dw, w_kv, w_kv_dw, w_proj, w_fuse, b_fuse,
                  temperature):
    """Host-side weight composition (numpy)."""
    w_pos = w_pos[:, :, 0, 0]      # [192,192]
    w_qv = w_qv[:, :, 0, 0]        # [192,96]
    w_kv = w_kv[:, :, 0, 0]        # [192,96]
    w_proj = w_proj[:, :, 0, 0]    # [192,96]
    dwq = w_qv_dw[:, 0].reshape(2 * C, 9)   # [192,9]
    dwk = w_kv_dw[:, 0].reshape(2 * C, 9)   # [192,9]

    out = {}
    # conv groups: A_t[i,o] = W[o,i] * dw[o,t]
    for nm, Wg, dwg in (("q", w_qv[:C], dwq[:C]), ("vx", w_qv[C:], dwq[C:]),
                        ("k", w_kv[:C], dwk[:C]), ("vy", w_kv[C:], dwk[C:])):
        A = np.einsum('oi,ot->tio', Wg, dwg)
        p, q, r = _group_weights(A)
        out[f"wP_{nm}"], out[f"wQ_{nm}"], out[f"wR_{nm}"] = p, q, r
    # fuse groups: A_t[j,o] = w_fuse[o, 96*half + j, t]
    wfr = w_fuse.reshape(C, 2 * C, 9)
    for nm, half in (("f0", 0), ("f_", 1)):
        A = np.transpose(wfr[:, half * C:(half + 1) * C, :], (2, 1, 0))
        p, q, r = _group_weights(np.ascontiguousarray(A))
        out[f"wP_{nm}"], out[f"wQ_{nm}"], out[f"wR_{nm}"] = p, q, r

    out["wprojT"] = _bf16(w_proj.T)          # [96 c', 192 o]
    out["wposxT"] = _bf16(w_pos[:, :C].T)    # [96 i, 192 o]
    out["wposyT"] = _bf16(w_pos[:, C:].T)    # [96 i, 192 o]
    out["temp_row"] = _f32(np.repeat(temperature.reshape(HEADS), CH)
                           .reshape(C, 1))
    out["bfuse"] = _f32(b_fuse.reshape(C, 1))
    out["identf"] = _f32(np.eye(C))
    out["mask"] = _f32(np.kron(np.eye(HEADS), np.ones((CH, CH))) * 30000.0
                       - 30000.0)
    return out


def _dummy_out(tc, nc, out_d):
    with tc.tile_pool(name="dummy", bufs=2) as dp:
        for mt, (o0, osz) in enumerate(((0, 128), (128, 64))):
            for cb in range(NCB):
                t = dp.tile([osz, NCHUNK], F32, tag=f"d{mt}")
                nc.vector.memset(t[:], 0.0)
                nc.sync.dma_start(
                    out=out_d[o0:o0 + osz, cb * NCHUNK:(cb + 1) * NCHUNK],
                    in_=t[:])


def _build_nc(repeat=1):
    nc = bacc.Bacc(None, name="cg_msa")

    x_d = nc.dram_tensor("x", [C, N], F32, kind="ExternalInput")
    y_d = nc.dram_tensor("y", [C, N], F32, kind="ExternalInput")
    w_d = {}
    wspec = []
    for g in ("q", "vx", "k", "vy", "f0", "f_"):
        wspec += [(f"wP_{g}", [128, 3 * C], BF16),
                  (f"wQ_{g}", [128, 3 * C], BF16),
                  (f"wR_{g}", [C, C], BF16)]
    wspec += [("wprojT", [C, 2 * C], BF16), ("wposxT", [C, 2 * C], BF16),
              ("wposyT", [C, 2 * C], BF16), ("temp_row", [C, 1], F32),
              ("bfuse", [C, 1], F32), ("identf", [C, C], F32),
              ("mask", [C, C], F32)]
    for nm, shp, dt in wspec:
        w_d[nm] = nc.dram_tensor(nm, shp, dt, kind="ExternalInput")
    out_d = nc.dram_tensor("out", [2 * C, N], F32, kind="ExternalOutput")
    v_d = nc.dram_tensor("vtmp", [C, N], BF16, kind="Internal")

    with tile.TileContext(nc) as tc:
        for rep in range(repeat):
            _emit(tc, nc, x_d, y_d, w_d, out_d, v_d, warmup=(rep == 0))
    nc.finalize()
    return nc


def _ring_runs(r0, r1):
    """Split canvas rows [r0, r1] into contiguous phys runs of the mod-16
    ring.  Returns (phys_start, src_row_start, n) tuples."""
    runs = []
    r = r0
    while r <= r1:
        p = r % RRING
        n = min(RRING - p, r1 - r + 1)
        runs.append((p, r, n))
        r += n
    return runs


def _emit(tc, nc, x_d, y_d, w_d, out_d, v_d, warmup=True):
    from contextlib import ExitStack
    ctx = ExitStack()
    dma_rr = [nc.sync, nc.scalar, nc.gpsimd]
    rrctr = [0]

    def deng():
        e = dma_rr[rrctr[0] % 3]
        rrctr[0] += 1
        return e

    with ctx:
        const = ctx.enter_context(tc.tile_pool(name="const", bufs=1))
        canv = ctx.enter_context(tc.tile_pool(name="canv", bufs=1))
        stats = ctx.enter_context(tc.tile_pool(name="stats", bufs=1))
        cstk = ExitStack()  # phase-C-scoped SBUF pools (rings, io, vstg)
        ringp = cstk.enter_context(tc.tile_pool(name="ringp", bufs=1))

        # ---- weights to SBUF ----
        wsb = {}
        for wi, (nm, t_d) in enumerate(w_d.items()):
            t = const.tile(list(t_d.shape), t_d.dtype, tag=f"w_{nm}")
            (nc.sync if wi % 2 == 0 else nc.scalar).dma_start(
                out=t[:], in_=t_d[:])
            wsb[nm] = t

        # ---- canvases (128 partitions: 96 base + 32 aug) + rings ----
        cvs = {}
        for s in ("x", "y", "v0", "v_"):
            cv = canv.tile([128, CANV], BF16, tag=f"cv_{s}")
            # zero borders on all 128 partitions (covers aug block too)
            nc.gpsimd.memset(cv[:, 0:WC], 0.0)
            nc.gpsimd.memset(cv[:, (H + 1) * WC:CANV], 0.0)
            side = cv[:].rearrange("p (r c) -> p r c", c=WC)
            nc.gpsimd.memset(side[:, 0:H + 1, W + 1:W + 2], 0.0)
            nc.gpsimd.memset(side[:, 1:H + 2, 0:1], 0.0)
            qr = ringp.tile([128, QRING * W], BF16, tag=f"qr_{s}")
            rr = ringp.tile([C, RRING * W], BF16, tag=f"rr_{s}")
            nc.gpsimd.memset(qr[:], 0.0)
            nc.gpsimd.memset(rr[:], 0.0)
            cvs[s] = (cv, qr, rr)

        nqp = stats.tile([C, NCB], F32, tag="nqp")
        nkp = stats.tile([C, NCB], F32, tag="nkp")

        # ---- ring write helpers (SBUF->SBUF DMA from canvas) ----
        def ring_write(s):
            """Propagate canvas rows it*4+1..+4 (fresh chunk rows) into the
            aug block + Q/R rings of source s."""
            def go(it):
                if it < 0 or it >= NCB:
                    return
                cv, qr, rr = cvs[s]
                cvr = cv[:].rearrange("p (r c) -> p r c", c=WC)
                qrv = qr[:].rearrange("p (r c) -> p r c", c=W)
                rrv = rr[:].rearrange("p (r c) -> p r c", c=W)
                r0 = it * RPC + 1
                # P-aug: ch0-31 shifted (0,+1) into partitions 96-127
                deng().dma_start(out=cvr[96:128, r0:r0 + RPC, 0:W],
                                 in_=cvr[0:32, r0:r0 + RPC, 1:1 + W])
                # Q-ring: ring row r <- canvas row r
                for pb, (sp, c0) in enumerate(((32, 1), (0, 2))):
                    for (ph, sr, n) in _ring_runs(r0, r0 + RPC - 1):
                        deng().dma_start(
                            out=qrv[64 * pb:64 * pb + 64, ph:ph + n, :],
                            in_=cvr[sp:sp + 64, sr:sr + n, c0:c0 + W])
                        # dup rows: phys 0,1 mirrored at 16,17
                        for d in range(2):
                            if ph <= d < ph + n:
                                deng().dma_start(
                                    out=qrv[64 * pb:64 * pb + 64,
                                            RRING + d:RRING + d + 1, :],
                                    in_=cvr[sp:sp + 64, sr + d - ph:
                                            sr + d - ph + 1, c0:c0 + W])
                # R-ring: block b ring row r <- canvas row r+b, cols 2..129
                for b in range(3):
                    lo = max(r0 - b, 0)
                    hi = r0 + RPC - 1 - b
                    if hi < lo:
                        continue
                    for (ph, rr_row, n) in _ring_runs(lo, hi):
                        deng().dma_start(
                            out=rrv[32 * b:32 * b + 32, ph:ph + n, :],
                            in_=cvr[64:96, rr_row + b:rr_row + b + n, 2:2 + W])
            return go

        ring_write_x = ring_write("x")
        ring_write_y = ring_write("y")
        ring_write_v0 = ring_write("v0")
        ring_write_v_ = ring_write("v_")

        def end_fixups(srcs):
            """Canvas row 129 (bottom pad, never loaded) maps to Q phys 1
            (dup 17) and R block2 phys 15 — re-zero the stale dups before
            chunk 31 reads them.  Must run after the last ring write that
            touches those rows (canvas row 113, chunk 28) for each source."""
            for s in srcs:
                cv, qr, rr = cvs[s]
                nc.gpsimd.memset(qr[:, (RRING + 1) * W:(RRING + 2) * W], 0.0)
                nc.gpsimd.memset(rr[64:96, (RRING - 1) * W:RRING * W], 0.0)

        # ---- canvas / ring views for matmuls ----
        def pview(cv, cb, sy):
            r = cv[:].rearrange("p (r c) -> p r c", c=WC)
            return r[:, cb * RPC + sy:cb * RPC + sy + RPC, 0:W]

        def bview(cv, cb, dy, dx, p=C):
            r = cv[:].rearrange("p (r c) -> p r c", c=WC)
            return r[0:p, cb * RPC + dy:cb * RPC + dy + RPC, dx:dx + W]

        def qview(qr, cb, sy):
            ph = (cb * RPC + sy) % RRING
            r = qr[:].rearrange("p (r c) -> p r c", c=W)
            return r[:, ph:ph + RPC, :]

        def rview(rr, cb):
            ph = (cb * RPC) % RRING
            r = rr[:].rearrange("p (r c) -> p r c", c=W)
            return r[0:C, ph:ph + RPC, :]

        def tap_mms(ps, g, s, cb, start=True, stop=True):
            """Accumulate the 9-tap folded conv for group g (weights) reading
            source s (canvas+rings) into psum ps: the 7-MM P/Q/R cover."""
            cv, qr, rr = cvs[s]
            for sy in range(3):
                nc.tensor.matmul(ps, wsb[f"wP_{g}"][:, sy * C:(sy + 1) * C],
                                 pview(cv, cb, sy),
                                 start=(start and sy == 0), stop=False)
            for sy in range(3):
                nc.tensor.matmul(ps, wsb[f"wQ_{g}"][:, sy * C:(sy + 1) * C],
                                 qview(qr, cb, sy),
                                 start=False, stop=False)
            nc.tensor.matmul(ps, wsb[f"wR_{g}"][:],
                             rview(rr, cb), start=False, stop=stop)

        # ---- phase B: stream x,y in, convert to fp16 canvases ----
        io = cstk.enter_context(tc.tile_pool(name="io", bufs=6))

        def load_chunk(cb):
            if cb >= NCB:
                return
            for si, (src_d, s) in enumerate(((x_d, "x"), (y_d, "y"))):
                tin = io.tile([C, NCHUNK], F32, tag="in")
                eng = nc.sync if si == 0 else nc.scalar
                eng.dma_start(
                    out=tin[:], in_=src_d[:, cb * NCHUNK:(cb + 1) * NCHUNK])
                dst = bview(cvs[s][0], cb, 1, 1)
                nc.vector.tensor_copy(
                    dst, tin[:].rearrange("p (r c) -> p r c", c=W))

        for cb in range(4):
            load_chunk(cb)

        # PE warmup during the startup bubble
        if warmup:
            with tc.tile_pool(name="warm", bufs=1) as warm, \
                 tc.tile_pool(name="warmps", bufs=1, space="PSUM") as warmps:
                wsc = warm.tile([C, NCHUNK], BF16)
                nc.vector.memset(wsc[:], 0.0)
                wps = warmps.tile([C, NCHUNK], F32)
                for _ in range(16):
                    nc.tensor.matmul(wps[:], wsc[:, 0:C], wsc[:],
                                     start=True, stop=True)

        # ---- phase C ----
        gctx = ExitStack()
        gpool = gctx.enter_context(tc.tile_pool(name="gps", bufs=1,
                                                space="PSUM"))
        g_ps = gpool.tile([C, C], F32)  # Gqk

        vstg_pool = cstk.enter_context(tc.tile_pool(name="vstg", bufs=3))

        def fuse_chunk(psE, cb):
            if cb < 0 or cb >= NCB:
                return
            ps = psE.tile([C, NCHUNK], F32, tag="v")
            tap_mms(ps[:], "f0", "v0", cb, start=True, stop=False)
            tap_mms(ps[:], "f_", "v_", cb, start=False, stop=True)
            vs = vstg_pool.tile([C, NCHUNK], BF16, tag="vs")
            nc.scalar.activation(
                vs[:], ps[:], mybir.ActivationFunctionType.Identity,
                bias=wsb["bfuse"][:], scale=1.0)
            nc.gpsimd.dma_start(
                out=v_d[:, cb * NCHUNK:(cb + 1) * NCHUNK], in_=vs[:])

        with tc.tile_pool(name="psC", bufs=5, space="PSUM") as psC, \
             tc.tile_pool(name="psE", bufs=2, space="PSUM") as psE, \
             tc.tile_pool(name="stC", bufs=6) as stC, \
             tc.tile_pool(name="sqp", bufs=2) as sqp, \
             tc.tile_pool(name="stT", bufs=3) as stT:

            def gram_chunk(tps, cb):
                if tps is None:
                    return
                for j in range(RPC):
                    st = (cb == 0 and j == 0)
                    sp = (cb == NCB - 1 and j == RPC - 1)
                    nc.tensor.matmul(
                        g_ps[:], tps[:, j, 0, :], tps[:, j, 1, :],
                        start=st, stop=sp, skip_group_check=True)

            def qkv_chunk(cb):
                """tap-MM + evac for chunk cb; returns tps tile."""
                if cb < 0 or cb >= NCB:
                    return None
                outs_sb = {}
                for side, (s, gqk, gv) in enumerate(
                        (("x", "q", "vx"), ("y", "k", "vy"))):
                    ps = psC.tile([C, NCHUNK], F32, tag="qv")
                    tap_mms(ps[:], gqk, s, cb)
                    sb = stC.tile([C, NCHUNK], BF16, tag="qk")
                    nc.scalar.copy(out=sb[:], in_=ps[:])
                    sq = sqp.tile([C, NCHUNK], BF16, tag="sq")
                    npart = nqp if side == 0 else nkp
                    nc.scalar.activation(
                        sq[:], sb[:], mybir.ActivationFunctionType.Square,
                        accum_out=npart[:, cb:cb + 1])
                    outs_sb[side] = sb

                    psv = psC.tile([C, NCHUNK], F32, tag="qv")
                    tap_mms(psv[:], gv, s, cb)
                    cvv = cvs["v_"][0] if side == 0 else cvs["v0"][0]
                    nc.vector.tensor_copy(
                        bview(cvv, cb, 1, 1),
                        psv[:].rearrange("p (r c) -> p r c", c=W))

                tps = stT.tile([W, RPC, 2, C], BF16)
                for sd in range(2):
                    nc.scalar.dma_start_transpose(tps[:, :, sd, :],
                                                  outs_sb[sd][:])
                return tps

            # Pipeline: ring writes run a full iteration ahead of the matmuls
            # that consume them, so their DMA latency stays off the critical
            # path (each chunk's tap-MMs read one halo row produced by the
            # NEXT chunk's ring write).
            tps_hist = {}
            for it in range(NCB + 5):
                load_chunk(it + 4)
                ring_write_x(it)
                ring_write_y(it)
                if it == NCB - 2:
                    end_fixups(("x", "y"))
                tps_hist[it - 2] = qkv_chunk(it - 2)
                ring_write_v0(it - 3)
                ring_write_v_(it - 3)
                if it == NCB + 1:
                    end_fixups(("v0", "v_"))
                fuse_chunk(psE, it - 5)
                gram_chunk(tps_hist.get(it - 3), it - 3)
                tps_hist.pop(it - 4, None)
        cstk.close()

        # ---- phase D: norms, softmax, M1T ----
        smx = ctx.enter_context(tc.tile_pool(name="smx", bufs=1))
        with tc.tile_pool(name="psD", bufs=1, space="PSUM") as psD:
            g_sb = smx.tile([C, C], F32)
            nc.vector.tensor_copy(g_sb[:], g_ps[:])

            rr_ = {}
            for npart, nm in ((nqp, "q"), (nkp, "k")):
                nrm2 = smx.tile([C, 1], F32, tag=f"n{nm}")
                nc.vector.tensor_reduce(
                    nrm2[:], npart[:], axis=mybir.AxisListType.X,
                    op=mybir.AluOpType.add)
                nrm = smx.tile([C, 1], F32, tag=f"s{nm}")
                nc.scalar.sqrt(nrm[:], nrm2[:])
                nc.vector.tensor_scalar_max(nrm[:], nrm[:], 1e-12)
                rinv = smx.tile([C, 1], F32, tag=f"r{nm}")
                nc.vector.reciprocal(rinv[:], nrm[:])
                rr_[nm] = rinv
            nc.vector.tensor_tensor(
                rr_["q"][:], rr_["q"][:], wsb["temp_row"][:],
                mybir.AluOpType.mult)

            rows = {}
            for nm in ("q", "k"):
                rp = psD.tile([1, C], F32, tag="row")
                nc.tensor.transpose(rp[:], rr_[nm][:], wsb["identf"][:])
                rs = smx.tile([1, C], F32, tag=f"row{nm}")
                nc.vector.tensor_copy(rs[:], rp[:])
                rows[nm] = rs
            r_ps = psD.tile([C, C], F32, tag="R")
            nc.tensor.matmul(r_ps[:], rows["q"][:], rows["k"][:])
            logits = smx.tile([C, C], F32)
            nc.vector.tensor_tensor(
                logits[:], g_sb[:], r_ps[:], mybir.AluOpType.mult)
            nc.vector.tensor_tensor(
                logits[:], logits[:], wsb["mask"][:], mybir.AluOpType.add)

            mx = smx.tile([C, 1], F32)
            nc.vector.tensor_reduce(
                mx[:], logits[:], axis=mybir.AxisListType.X,
                op=mybir.AluOpType.max, negate=True)
            e = smx.tile([C, C], F32)
            nc.scalar.activation(
                e[:], logits[:], mybir.ActivationFunctionType.Exp,
                bias=mx[:], scale=1.0)
            s_ = smx.tile([C, 1], F32)
            nc.vector.tensor_reduce(
                s_[:], e[:], axis=mybir.AxisListType.X, op=mybir.AluOpType.add)
            rs = smx.tile([C, 1], F32)
            nc.vector.reciprocal(rs[:], s_[:])
            a_sb = smx.tile([C, C], BF16)
            nc.scalar.mul(a_sb[:], e[:], rs[:])

            m1_ps = psD.tile([C, 2 * C], F32, tag="m1")
            nc.tensor.matmul(m1_ps[:], a_sb[:], wsb["wprojT"][:])
            m1T = smx.tile([C, 2 * C], BF16)
            nc.vector.tensor_copy(m1T[:], m1_ps[:])
        gctx.close()

        # ---- phase F: out = M1 @ v + W_pos @ [x;y] ----
        with tc.tile_pool(name="psF", bufs=4, space="PSUM") as psF, \
             tc.tile_pool(name="vin", bufs=4) as vin, \
             tc.tile_pool(name="ostg", bufs=6) as ostg:
            vin_t = {}

            def vload(cb):
                if cb >= NCB:
                    return
                t = vin.tile([C, NCHUNK], BF16, tag="vin")
                nc.gpsimd.dma_start(
                    out=t[:], in_=v_d[:, cb * NCHUNK:(cb + 1) * NCHUNK])
                vin_t[cb] = t

            for cb in range(2):
                vload(cb)
            for cb in range(NCB):
                vload(cb + 2)
                vt = vin_t.pop(cb)
                for mt, (o0, osz) in enumerate(((0, 128), (128, 64))):
                    ps = psF.tile([osz, NCHUNK], F32, tag=f"o{mt}")
                    nc.tensor.matmul(
                        ps[:], wsb["wposxT"][:, o0:o0 + osz],
                        bview(cvs["x"][0], cb, 1, 1),
                        start=True, stop=False)
                    nc.tensor.matmul(
                        ps[:], wsb["wposyT"][:, o0:o0 + osz],
                        bview(cvs["y"][0], cb, 1, 1),
                        start=False, stop=False)
                    nc.tensor.matmul(
                        ps[:], m1T[:, o0:o0 + osz], vt[:],
                        start=False, stop=True)
                    osb = ostg.tile([osz, NCHUNK], F32, tag=f"os{mt}")
                    if mt == 0:
                        nc.scalar.copy(out=osb[:], in_=ps[:])
                    else:
                        nc.vector.tensor_copy(osb[:], ps[:])
                    oeng = nc.sync if cb % 2 == 0 else nc.scalar
                    oeng.dma_start(
                        out=out_d[o0:o0 + osz,
                                  cb * NCHUNK:(cb + 1) * NCHUNK],
                        in_=osb[:])


_NC_CACHE = None


def kernel(x, y, w_pos, w_qv, w_qv_dw, w_kv, w_kv_dw, w_proj, w_fuse, b_fuse,
           temperature):
    global _NC_CACHE, LAST_RESULTS
    x = _f32(np.asarray(x))
    y = _f32(np.asarray(y))
    wts = _prep_weights(
        np.asarray(w_pos, np.float32), np.asarray(w_qv, np.float32),
        np.asarray(w_qv_dw, np.float32), np.asarray(w_kv, np.float32),
        np.asarray(w_kv_dw, np.float32), np.asarray(w_proj, np.float32),
        np.asarray(w_fuse, np.float32), np.asarray(b_fuse, np.float32),
        np.asarray(temperature, np.float32))

    if _NC_CACHE is None:
        _NC_CACHE = _build_nc()
    nc = _NC_CACHE

    in_maps = []
    for core in range(B):
        m = {"x": np.ascontiguousarray(x[core].reshape(C, N)),
             "y": np.ascontiguousarray(y[core].reshape(C, N))}
        m.update(wts)
        in_maps.append(m)

    res = run_bass_kernel_spmd(nc, in_maps, core_ids=list(range(B)),
                               trace=TRACE)
    LAST_RESULTS = res
    out = np.stack([np.asarray(r["out"]) for r in res.results])
    return out.reshape(B, 2 * C, H, W).astype(np.float32)


if __name__ == "__main__":
    print("built nc ok" if _build_nc() else "")
